# revision 1
# baseline (speedup 1.0000x reference)
"""Trainium2 Bass kernel for nn_ContextualNodeModel (GNN message passing).

Strategy: edge-parallel sharding by destination-node ownership. Nodes are
assigned to the 8 cores (6250 each, 49 chunks of 128 slots). Every
aggregation contribution (fwd keyed by `future`, bwd keyed by `past`, frame
keyed by `early` and separately by `later`) is routed to the core owning its
destination node, so each core computes the full aggregate rows for its own
nodes and runs the total-flow MLP locally -- no collectives.

On device, per (list, chunk): remote/local endpoint features are gathered
from bf16 tables with dma_gather(transpose=True), which lands them directly
in [feature, edge] layout for the PE. The edge MLP layer-1 is computed as
hT[h,e] = sum_k W1[k,h]^T X^T[k,e] (+attr term), relu+bias on ACT, layer-2
as F[e,f], and the segment-sum is a matmul with a host-derived one-hot S:
aggT[f,n] += F^T S accumulated in PSUM per chunk. Pad edge slots carry an
out-of-range S value so they contribute nothing. The per-(list,chunk) tile
counts are the max over the 8 cores so one program serves all cores (SPMD).
"""
import os
import sys

sys.path.insert(0, "/opt/trn_rl_repo")

import numpy as np
import ml_dtypes

N_NODES = 50000
N_CORES = 8
NODES_PER_CORE = N_NODES // N_CORES          # 6250
CHUNK = 128
N_CHUNKS = (NODES_PER_CORE + CHUNK - 1) // CHUNK   # 49
SLOTS_PER_CORE = N_CHUNKS * CHUNK            # 6272
D = 256
D_EDGE = 32
D_F = 128
PAD_REL = -1000.0
LISTS = ("fwd", "bwd", "frE", "frL")
SEC = {"fwd": 0, "frE": 1, "frL": 1, "bwd": 2}
ROUND_TILES = 4                               # 512 edges per PSUM round

LAST_RESULTS = {}                             # stash for test harness


# ----------------------------------------------------------------- planning
def _assign_nodes(deg):
    """deg [4, N]: per-list destination degree. Balanced snake deal into
    8*49 bins, then node-count repair."""
    tot = deg.sum(axis=0).astype(np.int64)
    order = np.argsort(-tot, kind="stable")
    n_bins = N_CORES * N_CHUNKS
    idx = np.arange(N_NODES)
    rounds = idx // n_bins
    pos = idx % n_bins
    bins = np.where(rounds % 2 == 0, pos, n_bins - 1 - pos)
    assign = np.empty(N_NODES, np.int32)
    assign[order] = bins.astype(np.int32)

    capn = np.full(n_bins, CHUNK, np.int32)
    for c in range(N_CORES):
        capn[c * N_CHUNKS + N_CHUNKS - 1] = NODES_PER_CORE - (N_CHUNKS - 1) * CHUNK
    counts = np.bincount(assign, minlength=n_bins)
    over = {b for b in range(n_bins) if counts[b] > capn[b]}
    under = [b for b in range(n_bins) if counts[b] < capn[b]]
    if over:
        moved = []
        for v in order[::-1]:
            b = assign[v]
            if b in over and counts[b] > capn[b]:
                counts[b] -= 1
                moved.append(v)
        ui = 0
        for v in moved:
            while counts[under[ui]] >= capn[under[ui]]:
                ui += 1
            assign[v] = under[ui]
            counts[under[ui]] += 1

    node_perm = np.full((N_CORES, SLOTS_PER_CORE), -1, np.int64)
    fill = np.zeros(n_bins, np.int32)
    for v in range(N_NODES):
        b = assign[v]
        c, ch = divmod(b, N_CHUNKS)
        node_perm[c, ch * CHUNK + fill[b]] = v
        fill[b] += 1
    return node_perm


def _wrap_idx16(a):
    n = len(a)
    assert n % 16 == 0
    assert a.min(initial=0) >= 0 and a.max(initial=0) < 32768
    w = a.reshape(n // 16, 16).T.astype(np.int16)
    return np.tile(w, (8, 1))                 # [128, n/16]


def _build_plan(edge_index, same_frame_edge_index):
    ei = np.asarray(edge_index)
    fi = np.asarray(same_frame_edge_index)
    past, future = ei[0].astype(np.int64), ei[1].astype(np.int64)
    early, later = fi[0].astype(np.int64), fi[1].astype(np.int64)
    lists = {"fwd": (future, past, 0), "bwd": (past, future, 0),
             "frE": (early, later, 1), "frL": (later, early, 1)}

    deg = np.zeros((4, N_NODES), np.int32)
    for i, L in enumerate(LISTS):
        deg[i] = np.bincount(lists[L][0], minlength=N_NODES)
    node_perm = _assign_nodes(deg)

    node_core = np.empty(N_NODES, np.int32)
    node_slot = np.empty(N_NODES, np.int32)
    for c in range(N_CORES):
        valid = node_perm[c] >= 0
        node_core[node_perm[c][valid]] = c
        node_slot[node_perm[c][valid]] = np.nonzero(valid)[0]

    plan = {"node_perm": node_perm, "T": {}, "lists": {L: [] for L in LISTS},
            "chunk_off": {}, "rtab_rows": {}}
    for L in LISTS:
        dst, src, _ab = lists[L]
        dc = node_core[dst]
        dslot = node_slot[dst]
        dchunk = dslot // CHUNK
        counts = np.zeros((N_CORES, N_CHUNKS), np.int64)
        np.add.at(counts, (dc, dchunk), 1)
        T = np.maximum(1, (counts.max(axis=0) + CHUNK - 1) // CHUNK)
        plan["T"][L] = T
        chunk_off = np.concatenate([[0], np.cumsum(T * CHUNK)])
        plan["chunk_off"][L] = chunk_off
        n_slots = int(chunk_off[-1])
        rmax = 1
        for c in range(N_CORES):
            sel = np.nonzero(dc == c)[0]
            ch = dchunk[sel]
            order = np.argsort(ch, kind="stable")
            sel, ch = sel[order], ch[order]
            within = np.zeros(len(sel), np.int64)
            if len(sel):
                brk = np.nonzero(np.diff(ch))[0] + 1
                starts = np.concatenate([[0], brk])
                lens = np.diff(np.concatenate([starts, [len(sel)]]))
                within = np.arange(len(sel)) - np.repeat(starts, lens)
            slotpos = chunk_off[ch] + within
            uniq, inv = np.unique(src[sel], return_inverse=True)
            rmax = max(rmax, len(uniq))
            ridx = np.zeros(n_slots, np.int64)
            ridx[slotpos] = inv
            lidx = np.zeros(n_slots, np.int64)
            lidx[slotpos] = node_slot[dst[sel]]
            rel = np.full(n_slots, PAD_REL, np.float32)
            rel[slotpos] = (node_slot[dst[sel]] % CHUNK).astype(np.float32)
            attr = np.full(n_slots, -1, np.int64)
            attr[slotpos] = sel
            plan["lists"][L].append(
                {"ridx": ridx, "lidx": lidx, "rel": rel, "attr": attr,
                 "rtab_ids": uniq, "n_slots": n_slots})
        plan["rtab_rows"][L] = rmax
    return plan


# ----------------------------------------------------------- input packing
def _pack_core_inputs(inputs, plan, c):
    bf16 = ml_dtypes.bfloat16
    x = np.asarray(inputs["x"], np.float32)
    ea = np.asarray(inputs["edge_attr"], np.float32)
    fa = np.asarray(inputs["same_frame_edge_attr"], np.float32)
    attr_src = {"fwd": ea, "bwd": ea, "frE": fa, "frL": fa}

    d = {}
    ltab = np.zeros((SLOTS_PER_CORE, D), np.float32)
    valid = plan["node_perm"][c] >= 0
    ltab[valid] = x[plan["node_perm"][c][valid]]
    d["ltab"] = ltab.astype(bf16)

    ridx_cols, lidx_cols, attr_cols, rel_cols = [], [], [], []
    for L in LISTS:
        lp = plan["lists"][L][c]
        rt = np.zeros((plan["rtab_rows"][L], D), np.float32)
        rt[: len(lp["rtab_ids"])] = x[lp["rtab_ids"]]
        d[f"rtab_{L}"] = rt.astype(bf16)
        ridx_cols.append(_wrap_idx16(lp["ridx"]))
        lidx_cols.append(_wrap_idx16(lp["lidx"]))
        at = np.zeros((lp["n_slots"], D_EDGE), np.float32)
        real = lp["attr"] >= 0
        at[real] = attr_src[L][lp["attr"][real]]
        attr_cols.append(at.T.astype(bf16))                      # [32, n]
        rel_cols.append(lp["rel"].reshape(-1, CHUNK).T.copy())   # [128, ntiles]
    d["ridx_all"] = np.concatenate(ridx_cols, axis=1)
    d["lidx_all"] = np.concatenate(lidx_cols, axis=1)
    d["attrT_all"] = np.ascontiguousarray(np.concatenate(attr_cols, axis=1))
    d["rel_all"] = np.ascontiguousarray(np.concatenate(rel_cols, axis=1).astype(np.float32))

    # ---- weights (same for all cores)
    W1 = {"fwd": inputs["Wf1"], "bwd": inputs["Wb1"], "frE": inputs["Wr1"], "frL": inputs["Wr1"]}
    W2 = {"fwd": inputs["Wf2"], "bwd": inputs["Wb2"], "frE": inputs["Wr2"], "frL": inputs["Wr2"]}
    b1 = {"fwd": inputs["bf1"], "bwd": inputs["bb1"], "frE": inputs["br1"], "frL": inputs["br1"]}
    b2 = {"fwd": inputs["bf2"], "bwd": inputs["bb2"], "frE": inputs["br2"], "frL": inputs["br2"]}
    Wloc = {"fwd": W1["fwd"][0:D], "bwd": W1["bwd"][0:D],
            "frE": W1["frE"][0:D], "frL": W1["frL"][D:2 * D]}
    Wrem = {"fwd": W1["fwd"][D:2 * D], "bwd": W1["bwd"][D:2 * D],
            "frE": W1["frE"][D:2 * D], "frL": W1["frL"][0:D]}
    Watt = {L: np.asarray(W1[L])[2 * D:] for L in LISTS}

    def pack_k(Ws):   # list of [256, 256] -> [128, nlists*2*256]
        out = np.zeros((128, len(Ws) * 2 * 256), np.float32)
        for i, W in enumerate(Ws):
            W = np.asarray(W, np.float32)
            for kb in range(2):
                out[:, (i * 2 + kb) * 256:(i * 2 + kb + 1) * 256] = W[kb * 128:(kb + 1) * 128]
        return out

    d["Wrem"] = pack_k([Wrem[L] for L in LISTS]).astype(bf16)
    d["Wloc"] = pack_k([Wloc[L] for L in LISTS]).astype(bf16)
    wa = np.zeros((D_EDGE, 4 * 256), np.float32)
    for i, L in enumerate(LISTS):
        wa[:, i * 256:(i + 1) * 256] = np.asarray(Watt[L], np.float32)
    d["Watt"] = wa.astype(bf16)
    w2 = np.zeros((128, 4 * 2 * 128), np.float32)
    for i, L in enumerate(LISTS):
        W = np.asarray(W2[L], np.float32)            # [256, 128]
        for hb in range(2):
            w2[:, (i * 2 + hb) * 128:(i * 2 + hb + 1) * 128] = W[hb * 128:(hb + 1) * 128]
    d["W2"] = w2.astype(bf16)
    b1p = np.zeros((128, 8), np.float32)
    for i, L in enumerate(LISTS):
        bb = np.asarray(b1[L], np.float32)
        for hb in range(2):
            b1p[:, i * 2 + hb] = bb[hb * 128:(hb + 1) * 128]
    d["b1"] = b1p
    b2p = np.zeros((128, 4 * 512), np.float32)
    for i, L in enumerate(LISTS):
        b2p[:, i * 512:(i + 1) * 512] = np.tile(np.asarray(b2[L], np.float32), 4)[None, :]
    d["b2bc"] = b2p
    wt1 = np.zeros((128, 3 * 512), np.float32)
    Wt1 = np.asarray(inputs["Wt1"], np.float32)      # [384, 512]
    for kb in range(3):
        wt1[:, kb * 512:(kb + 1) * 512] = Wt1[kb * 128:(kb + 1) * 128]
    d["Wt1"] = wt1.astype(bf16)
    wt2 = np.zeros((128, 4 * 256), np.float32)
    Wt2 = np.asarray(inputs["Wt2"], np.float32)      # [512, 256]
    for hb in range(4):
        wt2[:, hb * 256:(hb + 1) * 256] = Wt2[hb * 128:(hb + 1) * 128]
    d["Wt2"] = wt2.astype(bf16)
    bt1p = np.zeros((128, 4), np.float32)
    bt1 = np.asarray(inputs["bt1"], np.float32)
    for hb in range(4):
        bt1p[:, hb] = bt1[hb * 128:(hb + 1) * 128]
    d["bt1"] = bt1p
    d["bt2bc"] = np.tile(np.asarray(inputs["bt2"], np.float32)[None, :], (128, 1)).astype(np.float32)
    d["iota"] = np.tile(np.arange(CHUNK, dtype=np.float32)[None, :], (128, 1))
    return d


# ------------------------------------------------------------ bass program
def _build_bass(plan, shapes):
    import concourse.bacc as bacc
    import concourse.tile as tile
    import concourse.mybir as mybir
    from concourse import library_config

    bf = mybir.dt.bfloat16
    f32 = mybir.dt.float32
    i16 = mybir.dt.int16

    nc = bacc.Bacc("TRN2", target_bir_lowering=False)
    dr = {}
    for name, (shape, dt) in shapes.items():
        kind = "ExternalOutput" if name == "out" else "ExternalInput"
        dr[name] = nc.dram_tensor(name, list(shape), dt, kind=kind)

    T = plan["T"]
    chunk_off = plan["chunk_off"]
    list_slot_base = {}
    list_tile_base = {}
    sb_, tb_ = 0, 0
    for L in LISTS:
        list_slot_base[L] = sb_
        list_tile_base[L] = tb_
        sb_ += int(chunk_off[L][-1])
        tb_ += int(T[L].sum())

    with tile.TileContext(nc) as tc:
        with (
            tc.tile_pool(name="const", bufs=1) as cpool,
            tc.tile_pool(name="gx", bufs=2) as gxpool,
            tc.tile_pool(name="work", bufs=2) as wpool,
            tc.tile_pool(name="spool", bufs=3) as spool,
            tc.tile_pool(name="ps_hT", bufs=1, space="PSUM") as ps_hT,
            tc.tile_pool(name="ps_F", bufs=2, space="PSUM") as ps_F,
            tc.tile_pool(name="ps_agg", bufs=2, space="PSUM") as ps_agg,
            tc.tile_pool(name="ps_m2", bufs=2, space="PSUM") as ps_m2,
        ):
            nc.gpsimd.load_library(library_config.mlp)

            # resident constants
            def cload(name, dt):
                t = cpool.tile(list(shapes[name][0]), dt, tag=name)
                nc.sync.dma_start(t[:], dr[name][:])
                return t

            ridx_sb = cload("ridx_all", i16)
            lidx_sb = cload("lidx_all", i16)
            rel_sb = cload("rel_all", f32)
            Wrem_sb = cload("Wrem", bf)
            Wloc_sb = cload("Wloc", bf)
            Watt_sb = cload("Watt", bf)
            W2_sb = cload("W2", bf)
            b1_sb = cload("b1", f32)
            b2bc_sb = cload("b2bc", f32)
            Wt1_sb = cload("Wt1", bf)
            Wt2_sb = cload("Wt2", bf)
            bt1_sb = cload("bt1", f32)
            bt2bc_sb = cload("bt2bc", f32)
            iota_sb = cload("iota", f32)

            li = {L: i for i, L in enumerate(LISTS)}

            for ch in range(N_CHUNKS):
                aggT = ps_agg.tile([128, 3, 128], f32, tag="aggT")
                sec_first = {0: True, 1: True, 2: True}
                n_sec_tiles = {0: int(T["fwd"][ch]), 1: int(T["frE"][ch] + T["frL"][ch]),
                               2: int(T["bwd"][ch])}
                sec_done = {0: 0, 1: 0, 2: 0}

                for L in LISTS:
                    iL = li[L]
                    Tc = int(T[L][ch])
                    ns = Tc * CHUNK
                    soff = list_slot_base[L] + int(chunk_off[L][ch])
                    toff = list_tile_base[L] + int(np.sum(T[L][:ch]))
                    sec = SEC[L]

                    xr = gxpool.tile([128, 2, ns], bf, tag="xr")
                    xl = gxpool.tile([128, 2, ns], bf, tag="xl")
                    at = gxpool.tile([32, ns], bf, tag="at")
                    nc.gpsimd.dma_gather(
                        xr[:], dr[f"rtab_{L}"][:], ridx_sb[:, soff // 16:(soff + ns) // 16],
                        ns, ns, D, transpose=True)
                    nc.gpsimd.dma_gather(
                        xl[:], dr["ltab"][:], lidx_sb[:, soff // 16:(soff + ns) // 16],
                        ns, ns, D, transpose=True)
                    nc.sync.dma_start(at[:], dr["attrT_all"][:, soff:soff + ns])

                    for r0 in range(0, Tc, ROUND_TILES):
                        rt = min(ROUND_TILES, Tc - r0)
                        rn = rt * CHUNK
                        e0 = r0 * CHUNK
                        hT = ps_hT.tile([128, 2, 512], f32, tag="hT")
                        for hb in range(2):
                            hcol = (iL * 2) * 256 + hb * 128
                            for kb in range(2):
                                nc.tensor.matmul(
                                    hT[:, hb, :rn],
                                    Wrem_sb[:, (iL * 2 + kb) * 256 + hb * 128:(iL * 2 + kb) * 256 + hb * 128 + 128],
                                    xr[:, kb, e0:e0 + rn],
                                    start=(kb == 0), stop=False)
                            for kb in range(2):
                                nc.tensor.matmul(
                                    hT[:, hb, :rn],
                                    Wloc_sb[:, (iL * 2 + kb) * 256 + hb * 128:(iL * 2 + kb) * 256 + hb * 128 + 128],
                                    xl[:, kb, e0:e0 + rn],
                                    start=False, stop=False)
                            nc.tensor.matmul(
                                hT[:, hb, :rn],
                                Watt_sb[:, iL * 256 + hb * 128:iL * 256 + hb * 128 + 128],
                                at[:, e0:e0 + rn],
                                start=False, stop=True)
                        hTs = wpool.tile([128, 2, 512], bf, tag="hTs")
                        for hb in range(2):
                            nc.scalar.activation(
                                hTs[:, hb, :rn], hT[:, hb, :rn],
                                mybir.ActivationFunctionType.Relu,
                                bias=b1_sb[:, iL * 2 + hb:iL * 2 + hb + 1])
                        Fp = ps_F.tile([128, 512], f32, tag="F")
                        for i in range(rt):
                            for hb in range(2):
                                nc.tensor.matmul(
                                    Fp[:, i * 128:(i + 1) * 128],
                                    hTs[:, hb, i * 128:(i + 1) * 128],
                                    W2_sb[:, (iL * 2 + hb) * 128:(iL * 2 + hb + 1) * 128],
                                    start=(hb == 0), stop=(hb == 1))
                        Fs = wpool.tile([128, 512], bf, tag="Fs")
                        nc.vector.tensor_tensor(
                            out=Fs[:, :rn], in0=Fp[:, :rn],
                            in1=b2bc_sb[:, iL * 512:iL * 512 + rn],
                            op=mybir.AluOpType.add)
                        for i in range(rt):
                            S = spool.tile([128, 128], bf, tag="S")
                            nc.vector.tensor_tensor(
                                out=S[:], in0=rel_sb[:, toff + r0 + i:toff + r0 + i + 1].to_broadcast([128, 128]),
                                in1=iota_sb[:], op=mybir.AluOpType.is_equal)
                            first = sec_first[sec]
                            sec_first[sec] = False
                            sec_done[sec] += 1
                            nc.tensor.matmul(
                                aggT[:, sec, :],
                                Fs[:, i * 128:(i + 1) * 128],
                                S[:],
                                start=first, stop=(sec_done[sec] == n_sec_tiles[sec]))

                # ---- total-flow MLP for this chunk
                aggTs = wpool.tile([128, 3, 128], bf, tag="aggTs")
                nc.vector.tensor_copy(out=aggTs[:], in_=aggT[:])
                h2 = ps_m2.tile([128, 4, 128], f32, tag="m2")
                for hb in range(4):
                    for kb in range(3):
                        nc.tensor.matmul(
                            h2[:, hb, :],
                            Wt1_sb[:, kb * 512 + hb * 128:kb * 512 + hb * 128 + 128],
                            aggTs[:, kb, :],
                            start=(kb == 0), stop=(kb == 2))
                h2s = wpool.tile([128, 4, 128], bf, tag="h2s")
                for hb in range(4):
                    nc.scalar.activation(
                        h2s[:, hb, :], h2[:, hb, :],
                        mybir.ActivationFunctionType.Relu,
                        bias=bt1_sb[:, hb:hb + 1])
                op = ps_m2.tile([128, 256], f32, tag="m2")
                for hb in range(4):
                    nc.tensor.matmul(
                        op[:], h2s[:, hb, :], Wt2_sb[:, hb * 256:(hb + 1) * 256],
                        start=(hb == 0), stop=(hb == 3))
                outs = wpool.tile([128, 256], f32, tag="outs")
                nc.vector.tensor_tensor(out=outs[:], in0=op[:], in1=bt2bc_sb[:],
                                        op=mybir.AluOpType.add)
                nc.sync.dma_start(dr["out"][ch], outs[:])

    nc.compile()
    return nc


# ----------------------------------------------------------------- kernel
def kernel(**inputs):
    import concourse.mybir as mybir
    from concourse.bass_utils import run_bass_kernel_spmd

    bf = mybir.dt.bfloat16
    f32 = mybir.dt.float32
    i16 = mybir.dt.int16

    plan = _build_plan(np.asarray(inputs["edge_index"]),
                       np.asarray(inputs["same_frame_edge_index"]))
    cores = [_pack_core_inputs(inputs, plan, c) for c in range(N_CORES)]

    shapes = {}
    for name, arr in cores[0].items():
        dt = {np.dtype(np.float32): f32, np.dtype(np.int16): i16,
              np.dtype(ml_dtypes.bfloat16): bf}[arr.dtype]
        shapes[name] = (arr.shape, dt)
    shapes["out"] = ((N_CHUNKS, 128, 256), f32)

    nc = _build_bass(plan, shapes)

    trace = bool(int(os.environ.get("GNN_TRACE", "0")))
    res = run_bass_kernel_spmd(nc, cores, core_ids=list(range(N_CORES)),
                               trace=trace)
    LAST_RESULTS["res"] = res

    out = np.zeros((N_NODES, 256), np.float32)
    for c in range(N_CORES):
        oc = np.asarray(res.results[c]["out"], np.float32).reshape(SLOTS_PER_CORE, 256)
        valid = plan["node_perm"][c] >= 0
        out[plan["node_perm"][c][valid]] = oc[valid]
    return out



# revision 2
# speedup vs baseline: 3.3176x; 3.3176x over previous
"""Trainium2 Bass kernel for nn_ContextualNodeModel (GNN message passing).

Strategy (v2): edge-parallel sharding by destination-node ownership with
HOST-SIDE pre-gathering. Nodes are dealt into 50 chunk-groups of 1000; within
each group a greedy 4-list degree balancer assigns nodes to the 8 cores (<=128
nodes per (core, chunk)), so per-(list, chunk) contribution counts are nearly
equal across cores and the shared SPMD program wastes little padding.

The v1 kernel gathered endpoint features on-device with gpsimd.dma_gather,
which made GPSIMD 91% busy and the critical path. v2 instead packs, on the
host, a PE-ready feature stream per core: for each (chunk, list) segment the
remote and local endpoint features land as [128, ns] k-tiles (xr_k0 xr_k1
xl_k0 xl_k1), streamed with one plain DMA per chunk. Edge MLP layer-1 runs as
W^T X^T accumulation in PSUM, relu+bias on ACT, layer-2 into [e, f] tiles, and
the segment-sum is a matmul with a one-hot S built on the vector engine from
per-slot destination rows. Slot counts are exact (16-aligned), not padded to
128; partial 128-tiles are neutralized by zero rows in S (hTs buffers are
memset once so stale columns stay finite). The total-flow MLP runs per chunk
on the aggregated sections, all node-local, no collectives.
"""
import os
import sys

sys.path.insert(0, "/opt/trn_rl_repo")

import numpy as np
import ml_dtypes

N_NODES = 50000
N_EDGES = 200000
N_FRAME_EDGES = 100000
N_CORES = 8
CHUNK = 128
N_CHUNKS = 50
GROUP = N_NODES // N_CHUNKS                  # 1000 nodes per chunk-group
SLOTS_PER_CORE = N_CHUNKS * CHUNK            # 6400
D = 256
D_EDGE = 32
PAD_REL = -1000.0
LISTS = ("fwd", "bwd", "frE", "frL")
SEC = {"fwd": 0, "frE": 1, "frL": 1, "bwd": 2}
ROUND_SLOTS = 512

LAST_RESULTS = {}                             # stash for test harness


# ----------------------------------------------------------------- planning
def _assign_nodes(deg):
    """deg [4, N]: per-list destination degree. Snake-deal nodes (by total
    degree) into 50 chunk-groups of 1000, then greedily split each group
    across the 8 cores balancing all 4 per-list degree sums."""
    tot = deg.sum(axis=0).astype(np.int64)
    order = np.argsort(-tot, kind="stable")
    idx = np.arange(N_NODES)
    rounds_i = idx // N_CHUNKS
    pos = idx % N_CHUNKS
    grp = np.where(rounds_i % 2 == 0, pos, N_CHUNKS - 1 - pos)
    group = np.empty(N_NODES, np.int32)
    group[order] = grp.astype(np.int32)

    node_perm = np.full((N_CORES, SLOTS_PER_CORE), -1, np.int64)
    degf = deg.astype(np.float64)
    for ch in range(N_CHUNKS):
        nodes = np.nonzero(group == ch)[0]
        nodes = nodes[np.argsort(-tot[nodes], kind="stable")]
        dn = degf[:, nodes].T                       # [g, 4]
        target = np.maximum(dn.sum(axis=0) / N_CORES, 1.0)
        load = np.zeros((N_CORES, 4))
        cnt = np.zeros(N_CORES, np.int64)
        for i in range(len(nodes)):
            cost = ((load + dn[i]) / target).max(axis=1) + 1e-4 * cnt
            cost[cnt >= CHUNK] = np.inf
            c = int(np.argmin(cost))
            node_perm[c, ch * CHUNK + cnt[c]] = nodes[i]
            load[c] += dn[i]
            cnt[c] += 1
    return node_perm


def _build_plan(edge_index, same_frame_edge_index):
    ei = np.asarray(edge_index)
    fi = np.asarray(same_frame_edge_index)
    past, future = ei[0].astype(np.int64), ei[1].astype(np.int64)
    early, later = fi[0].astype(np.int64), fi[1].astype(np.int64)
    # (dst, src, attr_base): attr id = attr_base + edge position
    lists = {"fwd": (future, past, 0), "bwd": (past, future, 0),
             "frE": (early, later, N_EDGES), "frL": (later, early, N_EDGES)}

    deg = np.zeros((4, N_NODES), np.int64)
    for i, L in enumerate(LISTS):
        deg[i] = np.bincount(lists[L][0], minlength=N_NODES)
    node_perm = _assign_nodes(deg)

    node_core = np.empty(N_NODES, np.int32)
    node_slot = np.empty(N_NODES, np.int32)
    for c in range(N_CORES):
        valid = node_perm[c] >= 0
        node_core[node_perm[c][valid]] = c
        node_slot[node_perm[c][valid]] = np.nonzero(valid)[0].astype(np.int32)

    plan = {"node_perm": node_perm, "ns_pad": {}, "tiles": {},
            "chunk_off": {}, "lists": {L: [] for L in LISTS}}
    for L in LISTS:
        dst, src, abase = lists[L]
        dc = node_core[dst]
        dslot = node_slot[dst]
        dchunk = dslot // CHUNK
        counts = np.zeros((N_CORES, N_CHUNKS), np.int64)
        np.add.at(counts, (dc, dchunk), 1)
        ns_pad = np.maximum(16, ((counts.max(axis=0) + 15) // 16) * 16)
        plan["ns_pad"][L] = ns_pad
        plan["tiles"][L] = (ns_pad + CHUNK - 1) // CHUNK
        chunk_off = np.concatenate([[0], np.cumsum(ns_pad)])
        plan["chunk_off"][L] = chunk_off
        n_slots = int(chunk_off[-1])
        for c in range(N_CORES):
            sel = np.nonzero(dc == c)[0]
            ch = dchunk[sel]
            order = np.argsort(ch, kind="stable")
            sel, ch = sel[order], ch[order]
            within = np.zeros(len(sel), np.int64)
            if len(sel):
                brk = np.nonzero(np.diff(ch))[0] + 1
                starts = np.concatenate([[0], brk])
                lens = np.diff(np.concatenate([starts, [len(sel)]]))
                within = np.arange(len(sel)) - np.repeat(starts, lens)
            slotpos = chunk_off[ch] + within
            srcid = np.full(n_slots, N_NODES, np.int64)          # pad -> zero row
            srcid[slotpos] = src[sel]
            dstid = np.full(n_slots, N_NODES, np.int64)
            dstid[slotpos] = dst[sel]
            attrid = np.full(n_slots, N_EDGES + N_FRAME_EDGES, np.int64)
            attrid[slotpos] = abase + sel
            rel = np.full(n_slots, PAD_REL, np.float32)
            rel[slotpos] = (node_slot[dst[sel]] % CHUNK).astype(np.float32)
            plan["lists"][L].append(
                {"srcid": srcid, "dstid": dstid, "attrid": attrid, "rel": rel})
    return plan


# ----------------------------------------------------------- input packing
def _pack_core_inputs(inputs, plan, c, xpad, attr_all):
    bf16 = ml_dtypes.bfloat16

    chunk_off = plan["chunk_off"]
    xtot = 4 * sum(int(chunk_off[L][-1]) for L in LISTS)
    stot = sum(int(chunk_off[L][-1]) for L in LISTS)
    ttot = int(sum(plan["tiles"][L].sum() for L in LISTS))

    d = {}
    xs = np.zeros((128, xtot), bf16)
    ats = np.zeros((D_EDGE, stot), bf16)
    rel = np.full((128, ttot), PAD_REL, np.float32)
    col = scol = tcol = 0
    for ch in range(N_CHUNKS):
        for L in LISTS:
            lp = plan["lists"][L][c]
            o0, o1 = int(chunk_off[L][ch]), int(chunk_off[L][ch + 1])
            ns = o1 - o0
            xr = xpad[lp["srcid"][o0:o1]]            # [ns, 256] f32
            xl = xpad[lp["dstid"][o0:o1]]
            xs[:, col:col + ns] = xr[:, :128].T; col += ns
            xs[:, col:col + ns] = xr[:, 128:].T; col += ns
            xs[:, col:col + ns] = xl[:, :128].T; col += ns
            xs[:, col:col + ns] = xl[:, 128:].T; col += ns
            ats[:, scol:scol + ns] = attr_all[lp["attrid"][o0:o1]].T
            scol += ns
            nt = int(plan["tiles"][L][ch])
            r = np.full((nt * CHUNK,), PAD_REL, np.float32)
            r[:ns] = lp["rel"][o0:o1]
            rel[:, tcol:tcol + nt] = r.reshape(nt, CHUNK).T
            tcol += nt
    d["xstream"] = xs
    d["attrs"] = ats
    d["rel_all"] = rel

    # ---- weights (same for all cores)
    W1 = {"fwd": inputs["Wf1"], "bwd": inputs["Wb1"], "frE": inputs["Wr1"], "frL": inputs["Wr1"]}
    W2 = {"fwd": inputs["Wf2"], "bwd": inputs["Wb2"], "frE": inputs["Wr2"], "frL": inputs["Wr2"]}
    b1 = {"fwd": inputs["bf1"], "bwd": inputs["bb1"], "frE": inputs["br1"], "frL": inputs["br1"]}
    b2 = {"fwd": inputs["bf2"], "bwd": inputs["bb2"], "frE": inputs["br2"], "frL": inputs["br2"]}
    Wloc = {"fwd": W1["fwd"][0:D], "bwd": W1["bwd"][0:D],
            "frE": W1["frE"][0:D], "frL": W1["frL"][D:2 * D]}
    Wrem = {"fwd": W1["fwd"][D:2 * D], "bwd": W1["bwd"][D:2 * D],
            "frE": W1["frE"][D:2 * D], "frL": W1["frL"][0:D]}
    Watt = {L: np.asarray(W1[L])[2 * D:] for L in LISTS}

    def pack_k(Ws):   # list of [256, 256] -> [128, nlists*2*256]
        out = np.zeros((128, len(Ws) * 2 * 256), np.float32)
        for i, W in enumerate(Ws):
            W = np.asarray(W, np.float32)
            for kb in range(2):
                out[:, (i * 2 + kb) * 256:(i * 2 + kb + 1) * 256] = W[kb * 128:(kb + 1) * 128]
        return out

    d["Wrem"] = pack_k([Wrem[L] for L in LISTS]).astype(bf16)
    d["Wloc"] = pack_k([Wloc[L] for L in LISTS]).astype(bf16)
    wa = np.zeros((D_EDGE, 4 * 256), np.float32)
    for i, L in enumerate(LISTS):
        wa[:, i * 256:(i + 1) * 256] = np.asarray(Watt[L], np.float32)
    d["Watt"] = wa.astype(bf16)
    w2 = np.zeros((128, 4 * 2 * 128), np.float32)
    for i, L in enumerate(LISTS):
        W = np.asarray(W2[L], np.float32)            # [256, 128]
        for hb in range(2):
            w2[:, (i * 2 + hb) * 128:(i * 2 + hb + 1) * 128] = W[hb * 128:(hb + 1) * 128]
    d["W2"] = w2.astype(bf16)
    b1p = np.zeros((128, 8), np.float32)
    for i, L in enumerate(LISTS):
        bb = np.asarray(b1[L], np.float32)
        for hb in range(2):
            b1p[:, i * 2 + hb] = bb[hb * 128:(hb + 1) * 128]
    d["b1"] = b1p
    b2p = np.zeros((128, 4 * 512), np.float32)
    for i, L in enumerate(LISTS):
        b2p[:, i * 512:(i + 1) * 512] = np.tile(np.asarray(b2[L], np.float32), 4)[None, :]
    d["b2bc"] = b2p
    wt1 = np.zeros((128, 3 * 512), np.float32)
    Wt1 = np.asarray(inputs["Wt1"], np.float32)      # [384, 512]
    for kb in range(3):
        wt1[:, kb * 512:(kb + 1) * 512] = Wt1[kb * 128:(kb + 1) * 128]
    d["Wt1"] = wt1.astype(bf16)
    wt2 = np.zeros((128, 4 * 256), np.float32)
    Wt2 = np.asarray(inputs["Wt2"], np.float32)      # [512, 256]
    for hb in range(4):
        wt2[:, hb * 256:(hb + 1) * 256] = Wt2[hb * 128:(hb + 1) * 128]
    d["Wt2"] = wt2.astype(bf16)
    bt1p = np.zeros((128, 4), np.float32)
    bt1 = np.asarray(inputs["bt1"], np.float32)
    for hb in range(4):
        bt1p[:, hb] = bt1[hb * 128:(hb + 1) * 128]
    d["bt1"] = bt1p
    d["bt2bc"] = np.tile(np.asarray(inputs["bt2"], np.float32)[None, :], (128, 1)).astype(np.float32)
    d["iota"] = np.tile(np.arange(CHUNK, dtype=np.float32)[None, :], (128, 1))
    return d


# ------------------------------------------------------------ bass program
def _build_bass(plan, shapes):
    import concourse.bacc as bacc
    import concourse.tile as tile
    import concourse.mybir as mybir

    bf = mybir.dt.bfloat16
    f32 = mybir.dt.float32

    nc = bacc.Bacc("TRN2", target_bir_lowering=False)
    dr = {}
    for name, (shape, dt) in shapes.items():
        kind = "ExternalOutput" if name == "out" else "ExternalInput"
        dr[name] = nc.dram_tensor(name, list(shape), dt, kind=kind)

    ns_pad = plan["ns_pad"]
    tiles = plan["tiles"]
    chunk_off = plan["chunk_off"]

    # per-chunk offsets into xstream / attrs / rel
    xoff = [0]
    soff = [0]
    toff = [0]
    for ch in range(N_CHUNKS):
        xc = sum(4 * int(ns_pad[L][ch]) for L in LISTS)
        sc = sum(int(ns_pad[L][ch]) for L in LISTS)
        tc = sum(int(tiles[L][ch]) for L in LISTS)
        xoff.append(xoff[-1] + xc)
        soff.append(soff[-1] + sc)
        toff.append(toff[-1] + tc)

    with tile.TileContext(nc) as tc:
        with (
            tc.tile_pool(name="const", bufs=1) as cpool,
            tc.tile_pool(name="gx", bufs=3) as gxpool,
            tc.tile_pool(name="work", bufs=2) as wpool,
            tc.tile_pool(name="spool", bufs=3) as spool,
            tc.tile_pool(name="ps_hT", bufs=2, space="PSUM") as ps_hT,
            tc.tile_pool(name="ps_F", bufs=1, space="PSUM") as ps_F,
            tc.tile_pool(name="ps_agg", bufs=2, space="PSUM") as ps_agg,
            tc.tile_pool(name="ps_m2", bufs=1, space="PSUM") as ps_m2,
        ):
            # resident constants
            def cload(name, dt):
                t = cpool.tile(list(shapes[name][0]), dt, tag=name)
                nc.sync.dma_start(t[:], dr[name][:])
                return t

            rel_sb = cload("rel_all", f32)
            Wrem_sb = cload("Wrem", bf)
            Wloc_sb = cload("Wloc", bf)
            Watt_sb = cload("Watt", bf)
            W2_sb = cload("W2", bf)
            b1_sb = cload("b1", f32)
            b2bc_sb = cload("b2bc", f32)
            Wt1_sb = cload("Wt1", bf)
            Wt2_sb = cload("Wt2", bf)
            bt1_sb = cload("bt1", f32)
            bt2bc_sb = cload("bt2bc", f32)
            iota_sb = cload("iota", f32)

            # zero-fill hTs ring so partial-tile reads past rn stay finite
            for _ in range(2):
                t0 = wpool.tile([128, 2, ROUND_SLOTS], bf, tag="hTs")
                nc.vector.memset(t0[:], 0.0)

            li = {L: i for i, L in enumerate(LISTS)}

            for ch in range(N_CHUNKS):
                xcols = xoff[ch + 1] - xoff[ch]
                scols = soff[ch + 1] - soff[ch]
                X = gxpool.tile([128, xcols], bf, tag="X")
                nc.sync.dma_start(X[:], dr["xstream"][:, xoff[ch]:xoff[ch + 1]])
                A = gxpool.tile([D_EDGE, scols], bf, tag="A")
                nc.sync.dma_start(A[:], dr["attrs"][:, soff[ch]:soff[ch + 1]])

                aggT = ps_agg.tile([128, 3, 128], f32, tag="aggT")
                sec_first = {0: True, 1: True, 2: True}
                n_sec_tiles = {0: int(tiles["fwd"][ch]),
                               1: int(tiles["frE"][ch] + tiles["frL"][ch]),
                               2: int(tiles["bwd"][ch])}
                sec_done = {0: 0, 1: 0, 2: 0}

                xbase = 0
                sbase = 0
                tbase = toff[ch]
                for L in LISTS:
                    iL = li[L]
                    ns = int(ns_pad[L][ch])
                    nt = int(tiles[L][ch])
                    sec = SEC[L]

                    for r0 in range(0, ns, ROUND_SLOTS):
                        rn = min(ROUND_SLOTS, ns - r0)           # slots in round
                        rt = (rn + CHUNK - 1) // CHUNK           # tiles in round
                        rc = rt * CHUNK                          # f-cols in round
                        hT = ps_hT.tile([128, 2, ROUND_SLOTS], f32, tag="hT")
                        for hb in range(2):
                            for kb in range(2):
                                nc.tensor.matmul(
                                    hT[:, hb, :rn],
                                    Wrem_sb[:, (iL * 2 + kb) * 256 + hb * 128:(iL * 2 + kb) * 256 + hb * 128 + 128],
                                    X[:, xbase + kb * ns + r0:xbase + kb * ns + r0 + rn],
                                    start=(kb == 0), stop=False)
                            for kb in range(2):
                                nc.tensor.matmul(
                                    hT[:, hb, :rn],
                                    Wloc_sb[:, (iL * 2 + kb) * 256 + hb * 128:(iL * 2 + kb) * 256 + hb * 128 + 128],
                                    X[:, xbase + (2 + kb) * ns + r0:xbase + (2 + kb) * ns + r0 + rn],
                                    start=False, stop=False)
                            nc.tensor.matmul(
                                hT[:, hb, :rn],
                                Watt_sb[:, iL * 256 + hb * 128:iL * 256 + hb * 128 + 128],
                                A[:, sbase + r0:sbase + r0 + rn],
                                start=False, stop=True)
                        hTs = wpool.tile([128, 2, ROUND_SLOTS], bf, tag="hTs")
                        for hb in range(2):
                            nc.scalar.activation(
                                hTs[:, hb, :rn], hT[:, hb, :rn],
                                mybir.ActivationFunctionType.Relu,
                                bias=b1_sb[:, iL * 2 + hb:iL * 2 + hb + 1])
                        Fp = ps_F.tile([128, ROUND_SLOTS], f32, tag="F")
                        for i in range(rt):
                            for hb in range(2):
                                nc.tensor.matmul(
                                    Fp[:, i * 128:(i + 1) * 128],
                                    hTs[:, hb, i * 128:(i + 1) * 128],
                                    W2_sb[:, (iL * 2 + hb) * 128:(iL * 2 + hb + 1) * 128],
                                    start=(hb == 0), stop=(hb == 1))
                        Fs = wpool.tile([128, ROUND_SLOTS], bf, tag="Fs")
                        nc.vector.tensor_tensor(
                            out=Fs[:, :rc], in0=Fp[:, :rc],
                            in1=b2bc_sb[:, iL * 512:iL * 512 + rc],
                            op=mybir.AluOpType.add)
                        for i in range(rt):
                            S = spool.tile([128, 128], bf, tag="S")
                            ti = tbase + r0 // CHUNK + i
                            nc.vector.tensor_tensor(
                                out=S[:], in0=rel_sb[:, ti:ti + 1].to_broadcast([128, 128]),
                                in1=iota_sb[:], op=mybir.AluOpType.is_equal)
                            first = sec_first[sec]
                            sec_first[sec] = False
                            sec_done[sec] += 1
                            nc.tensor.matmul(
                                aggT[:, sec, :],
                                Fs[:, i * 128:(i + 1) * 128],
                                S[:],
                                start=first, stop=(sec_done[sec] == n_sec_tiles[sec]))
                    xbase += 4 * ns
                    sbase += ns
                    tbase += nt

                # ---- total-flow MLP for this chunk
                aggTs = wpool.tile([128, 3, 128], bf, tag="aggTs")
                nc.vector.tensor_copy(out=aggTs[:], in_=aggT[:])
                h2 = ps_m2.tile([128, 4, 128], f32, tag="m2")
                for hb in range(4):
                    for kb in range(3):
                        nc.tensor.matmul(
                            h2[:, hb, :],
                            Wt1_sb[:, kb * 512 + hb * 128:kb * 512 + hb * 128 + 128],
                            aggTs[:, kb, :],
                            start=(kb == 0), stop=(kb == 2))
                h2s = wpool.tile([128, 4, 128], bf, tag="h2s")
                for hb in range(4):
                    nc.scalar.activation(
                        h2s[:, hb, :], h2[:, hb, :],
                        mybir.ActivationFunctionType.Relu,
                        bias=bt1_sb[:, hb:hb + 1])
                op = ps_m2.tile([128, 256], f32, tag="m2")
                for hb in range(4):
                    nc.tensor.matmul(
                        op[:], h2s[:, hb, :], Wt2_sb[:, hb * 256:(hb + 1) * 256],
                        start=(hb == 0), stop=(hb == 3))
                outs = wpool.tile([128, 256], f32, tag="outs")
                nc.vector.tensor_tensor(out=outs[:], in0=op[:], in1=bt2bc_sb[:],
                                        op=mybir.AluOpType.add)
                nc.sync.dma_start(dr["out"][ch], outs[:])

    nc.compile()
    return nc


# ----------------------------------------------------------------- kernel
def kernel(**inputs):
    import concourse.mybir as mybir
    from concourse.bass_utils import run_bass_kernel_spmd

    bf = mybir.dt.bfloat16
    f32 = mybir.dt.float32

    plan = _build_plan(np.asarray(inputs["edge_index"]),
                       np.asarray(inputs["same_frame_edge_index"]))

    x = np.asarray(inputs["x"], np.float32)
    xpad = np.vstack([x, np.zeros((1, D), np.float32)])
    ea = np.asarray(inputs["edge_attr"], np.float32)
    fa = np.asarray(inputs["same_frame_edge_attr"], np.float32)
    attr_all = np.vstack([ea, fa, np.zeros((1, D_EDGE), np.float32)])

    cores = [_pack_core_inputs(inputs, plan, c, xpad, attr_all)
             for c in range(N_CORES)]

    shapes = {}
    for name, arr in cores[0].items():
        dt = {np.dtype(np.float32): f32,
              np.dtype(ml_dtypes.bfloat16): bf}[arr.dtype]
        shapes[name] = (arr.shape, dt)
    shapes["out"] = ((N_CHUNKS, 128, 256), f32)

    nc = _build_bass(plan, shapes)

    trace = bool(int(os.environ.get("GNN_TRACE", "0")))
    res = run_bass_kernel_spmd(nc, cores, core_ids=list(range(N_CORES)),
                               trace=trace)
    LAST_RESULTS["res"] = res

    out = np.zeros((N_NODES, 256), np.float32)
    for c in range(N_CORES):
        oc = np.asarray(res.results[c]["out"], np.float32).reshape(SLOTS_PER_CORE, 256)
        valid = plan["node_perm"][c] >= 0
        out[plan["node_perm"][c][valid]] = oc[valid]
    return out
